# revision 1
# baseline (speedup 1.0000x reference)
"""Trainium2 Bass kernel for nn_LocalDenseCrossReadout (v3, bf16 pipelined).

Strategy:
- Data-parallel over batch: 8 batches -> 8 NeuronCores, one batch per core.
- Host-side (numpy, tiny): FiLM conditioning folded into per-batch q
  projection weights; LN affine + score scale folded; gate projections
  fused into the q/k projection weight matrices (544-wide outputs);
  v-projection bias folded into the output bias (softmax rows sum to 1);
  band mask sliced per q tile; all matmul operands pre-cast to bf16.
- Device kernel per core (single phase, everything resident in SBUF):
  LN chain kept short (bn_stats/aggr + rstd via pow(-0.5) on DVE, apply
  on Pool, XBAR DMA transpose dispatched from SP), bf16 projections with
  PSUM->SBUF bias copies on ScalarE, then banded attention per 128-row
  q tile, software-pipelined so the PE never waits on the softmax chain:
  gate logits -> tanh (sigmoid via 0.5+0.5*tanh(x/2), same act table as
  exp), scores + mask added via identity matmul into PSUM, exp,
  P=(1+t)*e with fused row-sum, P^T/oa^T via XBAR DMA transpose,
  attn@V, output projection.
"""

import sys

sys.path.insert(0, "/opt/trn_rl_repo")

import numpy as np

import concourse.bass as bass
import concourse.tile as tile
from concourse import bacc
from concourse import mybir
from concourse.bass_utils import run_bass_kernel_spmd
from concourse.masks import make_identity

DIM, QS, QT, KS, KT, WIN, B, RANK = 512, 64, 16, 256, 16, 4, 8, 32
Q = QS * QT  # 1024
K = KS * KT  # 4096
WINW = 768  # aligned kv window per 128-row q tile
NQT = Q // 128  # 8 q tiles
NKV = K // 128  # 32 kv tiles
F32 = mybir.dt.float32
BF16 = mybir.dt.bfloat16
F8 = mybir.dt.float8e4
FT = mybir.ActivationFunctionType
ALU = mybir.AluOpType
AX = mybir.AxisListType

# kv window start (aligned to 128) per q tile
WSTARTS = [0, 384, 896, 1408, 1920, 2432, 2944, 3328]


def build_bass():
    nc = bacc.Bacc("TRN2", target_bir_lowering=False)
    q = nc.dram_tensor("q", [Q, DIM], BF16, kind="ExternalInput")
    s = nc.dram_tensor("s", [K, DIM], BF16, kind="ExternalInput")
    wq = nc.dram_tensor("wq", [DIM, DIM + RANK], BF16, kind="ExternalInput")
    wk = nc.dram_tensor("wk", [DIM, DIM + RANK], BF16, kind="ExternalInput")
    wv = nc.dram_tensor("wv", [DIM, DIM], BF16, kind="ExternalInput")
    wo = nc.dram_tensor("wo", [DIM, DIM], BF16, kind="ExternalInput")
    rqt = nc.dram_tensor("rqt", [128, 5], F32, kind="ExternalInput")
    rkt = nc.dram_tensor("rkt", [128, 5], F32, kind="ExternalInput")
    bo2 = nc.dram_tensor("bo2", [1, DIM], BF16, kind="ExternalInput")
    bmask = nc.dram_tensor("bmask", [NQT, 128, WINW], BF16, kind="ExternalInput")
    out = nc.dram_tensor("out", [Q, DIM], F32, kind="ExternalOutput")

    with tile.TileContext(nc) as tc:
        with (
            tc.tile_pool(name="consts", bufs=1) as consts,
            tc.tile_pool(name="wts", bufs=1) as wts,
            tc.tile_pool(name="big", bufs=1) as big,
            tc.tile_pool(name="xin", bufs=3) as xin,
            tc.tile_pool(name="xnp", bufs=8) as xnp,
            tc.tile_pool(name="stats", bufs=8) as stats,
            tc.tile_pool(name="attn", bufs=3) as attn,
            tc.tile_pool(name="msks", bufs=1) as msks,
            tc.tile_pool(name="ps_s", bufs=2, space="PSUM") as ps_s,
            tc.tile_pool(name="ps_b", bufs=2, space="PSUM") as ps_b,
            tc.tile_pool(name="ps_t", bufs=2, space="PSUM") as ps_t,
        ):
            # ---------------- constants ----------------
            identb = consts.tile([128, 128], BF16)
            make_identity(nc, identb)
            eps = consts.tile([128, 1], F32)
            nc.vector.memset(eps, 1e-5)
            ones1 = consts.tile([1, 128], BF16)
            nc.vector.memset(ones1, 1.0)

            # weights as lhsT chunks: [128 (d_in in chunk c), c, d_out]
            def load_w(name, dram, n_out):
                t = wts.tile([128, 4, n_out], BF16, tag=name)
                nc.sync.dma_start(
                    out=t, in_=dram[:, :].rearrange("(c p) n -> p c n", p=128))
                return t

            # persistent activations (all bf16)
            qt_big = big.tile([128, 4, Q], BF16, tag="qt_big")    # xn_q^T
            st_big = big.tile([128, 4, K], BF16, tag="st_big")    # xn_s^T
            qpT = big.tile([128, 4, Q], F8, tag="qpT")            # q_p^T (fp8)
            gq = big.tile([32, Q], BF16, tag="gq")                # gate_q^T
            kT = big.tile([128, 4, K], F8, tag="kT")              # k_p^T (fp8)
            gk = big.tile([32, K], BF16, tag="gk")                # gate_k^T
            vb = big.tile([128, NKV, DIM], BF16, tag="vb")        # v_p rows

            # ---- stage 1 of LN pipeline: load one bank (512 rows) in one DMA
            def ln_load(src_dram, row0):
                x4 = xin.tile([128, 4, DIM], BF16, tag="x")
                nc.sync.dma_start(
                    out=x4, in_=src_dram[row0:row0 + 512, :].rearrange(
                        "(j p) n -> p j n", p=128))
                return x4

            # ---- stage 2: stats + apply + transpose into dst[:, :, col0:+128]
            def ln_tile(x, dst_big, col0):
                st6 = stats.tile([128, 6], F32, tag="st6")
                nc.vector.bn_stats(out=st6, in_=x)
                mv = stats.tile([128, 2], F32, tag="mv")
                nc.vector.bn_aggr(out=mv, in_=st6)
                sd = stats.tile([128, 1], F32, tag="sd")
                nc.scalar.activation(out=sd, in_=mv[:, 1:2],
                                     func=FT.Sqrt, bias=eps, scale=1.0)
                rstd = stats.tile([128, 1], F32, tag="rstd")
                nc.vector.reciprocal(out=rstd, in_=sd)
                nmr = stats.tile([128, 1], F32, tag="nmr")
                nc.vector.scalar_tensor_tensor(
                    out=nmr, in0=mv[:, 0:1], scalar=-1.0,
                    in1=rstd, op0=ALU.mult, op1=ALU.mult)
                xn = xnp.tile([128, DIM], BF16, tag="xn")
                nc.gpsimd.tensor_scalar(out=xn, in0=x, scalar1=rstd,
                                        scalar2=nmr, op0=ALU.mult, op1=ALU.add)
                # PE transpose: dst[dlow, c, col0+i] = xn[i, c*128+dlow]
                pt = ps_t.tile([128, 4, 128], BF16, tag="pt")
                for c in range(4):
                    nc.tensor.transpose(pt[:, c, :], xn[:, c * 128:(c + 1) * 128],
                                        identb)
                nc.vector.tensor_copy(dst_big[:, :, col0:col0 + 128], pt)

            # ---- projections for one bank of 512 rows (cols of the T layout)
            def proj_bank(w_sb, src_big, col0, dstT, dstG, r_sb):
                for m in range(5):
                    mw = 128 if m < 4 else RANK
                    mo = m * 128
                    pp = ps_s.tile([128, DIM], F32, tag="ps")
                    for c in range(4):
                        nc.tensor.matmul(pp[:mw, :], w_sb[:, c, mo:mo + mw],
                                         src_big[:, c, col0:col0 + 512],
                                         start=(c == 0), stop=(c == 3))
                    if m < 4:
                        nc.scalar.activation(out=dstT[:, m, col0:col0 + 512],
                                             in_=pp, func=FT.Identity,
                                             bias=r_sb[:, m:m + 1], scale=1.0)
                    else:
                        nc.scalar.activation(out=dstG[:, col0:col0 + 512],
                                             in_=pp[:RANK, :], func=FT.Identity,
                                             bias=r_sb[:RANK, 4:5], scale=1.0)

            # ---------------- LN + projections, one bank (512 rows) ahead ----
            # banks 0-1: q rows; banks 2-9: s rows
            def bank_src(bk):
                return (q, bk * 512) if bk < 2 else (s, (bk - 2) * 512)

            state = {}

            def attn_front(t):
                """gate logits, tanh, scores+mask, exp, P, P^T."""
                w0 = WSTARTS[t]
                qc = bass.ts(t, 128)
                gl = ps_b.tile([128, WINW], F32, tag="glsc")
                for n0, nn_ in ((0, 512), (512, 256)):
                    nc.tensor.matmul(gl[:, n0:n0 + nn_], gq[:, qc],
                                     gk[:, w0 + n0:w0 + n0 + nn_],
                                     start=True, stop=True)
                # 2*sigmoid(gl) = 1 + tanh(gl/2); factor 2 cancels in softmax
                tq = attn.tile([128, WINW], BF16, tag="tq")
                nc.scalar.activation(out=tq, in_=gl, func=FT.Tanh,
                                     bias=0.0, scale=0.5)
                sc = ps_b.tile([128, WINW], F32, tag="glsc")
                for n0, nn_ in ((0, 512), (512, 256)):
                    for c in (0, 2):
                        nc.tensor.matmul(sc[:, n0:n0 + nn_],
                                         qpT[:, c:c + 2, qc],
                                         kT[:, c:c + 2, w0 + n0:w0 + n0 + nn_],
                                         start=(c == 0), stop=False,
                                         perf_mode=mybir.MatmulPerfMode.DoubleRow)
                    nc.tensor.matmul(sc[:, n0:n0 + nn_], identb,
                                     msk_t[t][:, n0:n0 + nn_],
                                     start=False, stop=True)
                e = attn.tile([128, WINW], BF16, tag="e")
                nc.scalar.activation(out=e, in_=sc, func=FT.Exp, bias=0.0)
                P = attn.tile([128, WINW], BF16, tag="P")
                rsum = stats.tile([128, 1], F32, tag="rsum")
                nc.vector.scalar_tensor_tensor(
                    out=P, in0=tq, scalar=1.0, in1=e,
                    op0=ALU.add, op1=ALU.mult, accum_out=rsum)
                rinv = stats.tile([128, 1], F32, tag="rinv")
                nc.vector.reciprocal(out=rinv, in_=rsum)
                aT = attn.tile([128, 6, 128], BF16, tag="aT")
                nc.sync.dma_start_transpose(aT, P)
                state[t] = (aT, rinv)

            def attn_mid(t):
                """attn@V, normalize, oa^T."""
                w0 = WSTARTS[t]
                aT, rinv = state.pop(t)
                av = ps_s.tile([128, DIM], F32, tag="ps")
                for cc in range(6):
                    nc.tensor.matmul(av, aT[:, cc, :], vb[:, w0 // 128 + cc, :],
                                     start=(cc == 0), stop=(cc == 5))
                oa = attn.tile([128, DIM], BF16, tag="oa")
                nc.scalar.mul(oa, av, rinv)
                oaT = attn.tile([128, 4, 128], BF16, tag="oaT")
                nc.sync.dma_start_transpose(oaT, oa)
                state[(t, "oaT")] = oaT

            def attn_fin(t):
                """out projection + store."""
                qc = bass.ts(t, 128)
                oaT = state.pop((t, "oaT"))
                fin = ps_s.tile([128, DIM], F32, tag="ps")
                for c in range(4):
                    nc.tensor.matmul(fin, oaT[:, c, :], wo_sb[:, c, :],
                                     start=(c == 0), stop=False)
                nc.tensor.matmul(fin, ones1, bo2_sb, start=False, stop=True)
                ob = attn.tile([128, DIM], F32, tag="ob")
                nc.scalar.copy(ob, fin)
                nc.sync.dma_start(out=out[qc, :], in_=ob)

            # ---- unified schedule: LN+projections, with attention tiles
            # pumped in as soon as their kv window is projected
            need_bank = [(WSTARTS[t] + WINW + 511) // 512 - 1 for t in range(NQT)]
            prog = {"f": 0, "m": 0, "o": 0}

            def pump(done_kb):
                while prog["f"] < NQT and need_bank[prog["f"]] <= done_kb:
                    attn_front(prog["f"])
                    prog["f"] += 1
                    while prog["m"] < max(0, prog["f"] - 1):
                        attn_mid(prog["m"])
                        prog["m"] += 1
                    while prog["o"] < max(0, prog["m"] - 1):
                        attn_fin(prog["o"])
                        prog["o"] += 1

            # input banks first on the DMA queue so LN starts immediately;
            # weights and masks load behind them (needed later)
            pending = [ln_load(*bank_src(0)), ln_load(*bank_src(1))]
            rqt_sb = consts.tile([128, 5], F32)
            nc.sync.dma_start(out=rqt_sb, in_=rqt[:, :])
            rkt_sb = consts.tile([128, 5], F32)
            nc.sync.dma_start(out=rkt_sb, in_=rkt[:, :])
            bo2_sb = consts.tile([1, DIM], BF16)
            nc.sync.dma_start(out=bo2_sb, in_=bo2[:, :])
            wq_sb = load_w("wq", wq, DIM + RANK)
            wk_sb = load_w("wk", wk, DIM + RANK)
            wv_sb = load_w("wv", wv, DIM)
            wo_sb = load_w("wo", wo, DIM)
            mskall = msks.tile([128, NQT, WINW], BF16, tag="msk")
            nc.sync.dma_start(
                out=mskall,
                in_=bmask[:, :, :].rearrange("t p n -> p t n"))
            msk_t = [mskall[:, t, :] for t in range(NQT)]

            for bk in range(10):
                xq = pending.pop(0)
                if bk + 2 < 10:
                    pending.append(ln_load(*bank_src(bk + 2)))
                if bk < 2:
                    for j in range(4):
                        ln_tile(xq[:, j, :], qt_big, bk * 512 + j * 128)
                    proj_bank(wq_sb, qt_big, bk * 512, qpT, gq, rqt_sb)
                else:
                    kb = bk - 2
                    for j in range(4):
                        ln_tile(xq[:, j, :], st_big, kb * 512 + j * 128)
                    proj_bank(wk_sb, st_big, kb * 512, kT, gk, rkt_sb)
                    for j in range(4):
                        jj = kb * 4 + j
                        pv = ps_s.tile([128, DIM], F32, tag="ps")
                        for c in range(4):
                            nc.tensor.matmul(pv, st_big[:, c, jj * 128:(jj + 1) * 128],
                                             wv_sb[:, c, :], start=(c == 0), stop=(c == 3))
                        if j % 2 == 0:
                            nc.vector.tensor_copy(vb[:, jj, :], pv)
                        else:
                            nc.scalar.copy(vb[:, jj, :], pv)
                    pump(kb)
            while prog["m"] < NQT:
                attn_mid(prog["m"])
                prog["m"] += 1
            while prog["o"] < NQT:
                attn_fin(prog["o"])
                prog["o"] += 1

    if not nc.is_finalized():
        nc.finalize()
    return nc


_NC_CACHE = None


def _get_nc():
    global _NC_CACHE
    if _NC_CACHE is None:
        _NC_CACHE = build_bass()
    return _NC_CACHE


def _host_fold(inputs):
    f32 = np.float32
    bf16 = mybir.dt.np(BF16)
    scale = f32(DIM ** -0.5)
    sqr = f32(np.sqrt(RANK))
    ctx0 = np.asarray(inputs["ctx0"], f32)
    ctx1 = np.asarray(inputs["ctx1"], f32)
    pre = ctx0 @ inputs["Wc0"] + inputs["bc0"] + ctx1 @ inputs["Wc1"] + inputs["bc1"]
    pre = np.asarray(pre, f32)
    h = pre / (1.0 + np.exp(-pre))
    gbv = np.asarray(h @ inputs["Wf"] + inputs["bf"], f32)
    gamma, beta = gbv[:, :DIM], gbv[:, DIM:]

    qn_g = np.asarray(inputs["qn_g"], f32)
    qn_b = np.asarray(inputs["qn_b"], f32)
    kvn_g = np.asarray(inputs["kvn_g"], f32)
    kvn_b = np.asarray(inputs["kvn_b"], f32)
    Wq, bq = np.asarray(inputs["Wq"], f32), np.asarray(inputs["bq"], f32)
    Wk, bk = np.asarray(inputs["Wk"], f32), np.asarray(inputs["bk"], f32)
    Wv, bv = np.asarray(inputs["Wv"], f32), np.asarray(inputs["bv"], f32)
    Wo, bo = np.asarray(inputs["Wo"], f32), np.asarray(inputs["bo"], f32)
    Wgq = np.asarray(inputs["Wgq"], f32)
    Wgk = np.asarray(inputs["Wgk"], f32)
    mask = np.asarray(inputs["mask"], f32)

    # k path (batch-independent): LN affine folded; gate_k fused as extra cols
    WkS = Wk * kvn_g[:, None]
    rk = (kvn_b @ Wk + bk).astype(f32)
    wk_ext = np.concatenate([WkS, WkS @ Wgk], axis=1).astype(bf16)
    rkt = np.zeros((128, 5), f32)
    rkt[:, :4] = rk.reshape(4, 128).T
    rkt[:RANK, 4] = rk @ Wgk
    # v path: bias folded into output bias (attn rows sum to 1)
    WvS = (Wv * kvn_g[:, None]).astype(bf16)
    rv = (kvn_b @ Wv + bv).astype(f32)
    bo2 = (rv @ Wo + bo).reshape(1, DIM).astype(bf16)
    Wo_b = np.ascontiguousarray(Wo).astype(bf16)

    bmask = np.stack([mask[t * 128:(t + 1) * 128, w:w + WINW]
                      for t, w in enumerate(WSTARTS)])
    bmask = np.maximum(bmask, -1e30).astype(bf16)

    query = np.asarray(inputs["query"], f32).reshape(B, Q, DIM)
    source = np.asarray(inputs["source"], f32).reshape(B, K, DIM)

    in_maps = []
    for b in range(B):
        sg = qn_g * (1.0 + gamma[b])
        off = qn_b * (1.0 + gamma[b]) + beta[b]
        Wq_f = Wq * sg[:, None]
        rq_raw = (off @ Wq + bq).astype(f32)
        wq_ext = np.concatenate([Wq_f * scale, (Wq_f @ Wgq) / sqr], axis=1)
        rqt_b = np.zeros((128, 5), f32)
        rqt_b[:, :4] = (rq_raw * scale).reshape(4, 128).T
        rqt_b[:RANK, 4] = rq_raw @ Wgq / sqr
        in_maps.append({
            "q": query[b].astype(bf16),
            "s": source[b].astype(bf16),
            "wq": wq_ext.astype(bf16),
            "wk": wk_ext,
            "wv": WvS,
            "wo": Wo_b,
            "rqt": rqt_b,
            "rkt": rkt,
            "bo2": bo2,
            "bmask": bmask,
        })
    return in_maps


def kernel(**inputs):
    nc = _get_nc()
    in_maps = _host_fold(inputs)
    res = run_bass_kernel_spmd(nc, in_maps, core_ids=list(range(B)))
    out = np.stack([res.results[b]["out"] for b in range(B)])
    return out.reshape(B, QS, QT, DIM).astype(np.float32)


if __name__ == "__main__":
    build_bass()
    print("bass build OK")

